# revision 2
# baseline (speedup 1.0000x reference)
"""Trainium2 Bass kernel for nn_Eq1to3 (gnn_message_passing).

Reference computation:
    Y  = einsum('ndi,dsb->nsbi', x, coefs[:, :, :3])      # (n, s, 3, m)
    S  = einsum('nd,ds->ns', x.sum(-1), coefs[:, :, 3])   # (n, s)
    out[n,s,i,j,k] = Y0[n,s,i] + Y1[n,s,j] + Y2[n,s,k] + S[n,s] + bias[s]

Shapes: x (4, 16, 96) f32 -> out (4, 16, 96, 96, 96) f32 (~226.5 MB).
The contractions are tiny; the kernel is output-write bound.

Strategy (8 NeuronCores):
  * Shard (n, i): core c handles n = c//2, i in [48*(c%2), 48*(c%2)+48).
    Per-core output 7.08M elements - balanced, no collectives.
  * Host precomputes (microscopic contractions, fp32 exact):
        W[n, s, (j,k)] = Y1[n,s,j] + Y2[n,s,k] + S[n,s] + bias[s]   (i-free)
        A[n, s, i]     = Y0[n,s,i]
    and ships W as fp16 (the device output is fp16; see below).
  * Device tile layout: 128 partitions = (s: 16) x (i-chunk: 8), free dim =
    (j,k) = 9216.  big0 (fp16) holds W replicated 8x per s-row, built once
    by 8 zero-stride broadcast DMAs from the packed (128, 1152) fp16 w.
  * Per i-chunk t (6 total): big = big0 + a_t with the per-element adds
    split across TWO engines by clock ratio - DVE (tensor_scalar_add) on
    cols [0:4096], ACT (activation Identity w/ per-partition bias) on cols
    [4096:9216] - then each engine half is DMA'd to HBM from its own HWDGE
    ring (SP / ACT) as soon as its sub-ops finish.
  * fp16 output: halves HBM write bytes (14.2 MB/core).  Total device-side
    rounding error ~1e-3 relative (fp16 W storage + fp16 output rounding)
    vs the 2e-2 gate.  Host casts back to f32 on reassembly.

Measured on HW (in-NEFF replication slope, K=1 vs 201, interleaved
single-dispatch sampling): ~17-20 us/exec vs ~147 us for the previous
all-f32 single-engine version.  Only SP/ACT can host HWDGE rings; 1-queue
vs 2-queue f32 probes showed no queue-count scaling, while fp16 halves
bytes ~2.5x faster => the win is bytes + descriptor shape, not queues.
"""

import dataclasses
import sys

sys.path.insert(0, "/opt/trn_rl_repo")

import numpy as np

import concourse.bacc as bacc
import concourse.mybir as mybir
from concourse.tile import TileContext
from concourse.bass_utils import run_bass_kernel_spmd

N_BATCH = 4
IN_DIM = 16
OUT_DIM = 16
M = 96
JK = M * M  # 9216
N_CORES = 8
I_PER_CORE = 48  # one n, half of the i axis per core
I_CHUNK = 8  # 16 s * 8 i' = 128 partitions
N_CHUNKS = I_PER_CORE // I_CHUNK  # 6
PITCH = JK // I_CHUNK  # 1152: packed-W row length
C_DVE = 4096  # DVE's columns per chunk; ACT takes the rest (0.96 : 1.2 GHz)
DVE_SPLIT = 4  # sub-ops per engine half (finer deps -> earlier DMA starts)
ACT_SPLIT = 4

_PROGRAM_CACHE = {}


def _build_program(rep=1, serialize=False):
    """rep>1 replicates the kernel body in-NEFF for slope timing (test.py);
    serialize chains rep r+1's compute to a DRAM readback of rep r's last
    output chunk so repeats can neither overlap nor be elided.  The
    production program is rep=1, serialize=False."""
    nc = bacc.Bacc(None)
    # Packed W: row p = W[n, p//8, (p%8)*PITCH : (p%8+1)*PITCH]  (128, 1152)
    w_d = nc.dram_tensor("w", [128, PITCH], mybir.dt.float16, kind="ExternalInput")
    # A columns: a[p, t] = Y0 value for partition p = (s, i') in i-chunk t
    a_d = nc.dram_tensor("a", [128, N_CHUNKS], mybir.dt.float32, kind="ExternalInput")
    o_d = nc.dram_tensor(
        "o", [N_CHUNKS, OUT_DIM, I_CHUNK, JK], mybir.dt.float16, kind="ExternalOutput"
    )

    with TileContext(nc) as tc:
        with (
            tc.tile_pool(name="spool", bufs=2) as spool,
            tc.tile_pool(name="b0pool", bufs=1) as b0pool,
            tc.tile_pool(name="bigpool", bufs=4) as bigpool,
        ):
            big0 = b0pool.tile([128, JK], mybir.dt.float16)
            # Replicate: big0[p=(s,i'), e*PITCH+k'] = w[s*8+e, k'] for all
            # i'.  DRAM source AP [[PITCH*8, 16], [0, 8], [1, PITCH]] at
            # offset e*PITCH: the zero-stride middle dim re-reads each packed
            # W row for all 8 destination partitions of its s-group.
            for e in range(I_CHUNK):
                src = dataclasses.replace(
                    w_d[:],
                    offset=e * PITCH,
                    ap=[[PITCH * I_CHUNK, OUT_DIM], [0, I_CHUNK], [1, PITCH]],
                )
                eng = nc.sync if e % 2 == 0 else nc.scalar
                eng.dma_start(out=big0[:, e * PITCH : (e + 1) * PITCH], in_=src)

            for r in range(rep):
                a_sb = spool.tile([128, N_CHUNKS], mybir.dt.float32)
                nc.sync.dma_start(out=a_sb[:], in_=a_d[:])
                if serialize and r > 0:
                    # RAW through DRAM (timing rig only): rep r's compute
                    # consumes bytes DMA'd back from rep r-1's last output
                    # chunk.  Corrupts big0[:, 0:1]; values are irrelevant.
                    rb = spool.tile([128, 1], mybir.dt.float16)
                    nc.sync.dma_start(out=rb[:, 0:1], in_=o_d[5, :, 0, 0:8])
                    nc.vector.tensor_copy(out=big0[:, 0:1], in_=rb[:, 0:1])
                for t in range(N_CHUNKS):
                    big = bigpool.tile([128, JK], mybir.dt.float16)
                    a_t = a_sb[:, t : t + 1]
                    fs = C_DVE // DVE_SPLIT
                    for f in range(DVE_SPLIT):
                        sl = slice(f * fs, (f + 1) * fs)
                        nc.vector.tensor_scalar_add(
                            out=big[:, sl], in0=big0[:, sl], scalar1=a_t
                        )
                    fs2 = (JK - C_DVE) // ACT_SPLIT
                    for f in range(ACT_SPLIT):
                        sl = slice(C_DVE + f * fs2, C_DVE + (f + 1) * fs2)
                        nc.scalar.add(out=big[:, sl], in_=big0[:, sl], add=a_t)
                    nc.sync.dma_start(out=o_d[t, :, :, 0:C_DVE], in_=big[:, 0:C_DVE])
                    nc.scalar.dma_start(
                        out=o_d[t, :, :, C_DVE:JK], in_=big[:, C_DVE:JK]
                    )
    nc.compile()
    return nc


def _host_precompute(x, coefs, bias):
    x = np.asarray(x, dtype=np.float32)
    coefs = np.asarray(coefs, dtype=np.float32)
    bias = np.asarray(bias, dtype=np.float32)
    Y = np.einsum("ndi,dsb->nsbi", x, coefs[:, :, :3], optimize=True).astype(np.float32)
    S = np.einsum("nd,ds->ns", x.sum(axis=-1), coefs[:, :, 3], optimize=True).astype(
        np.float32
    )
    A = Y[:, :, 0, :]  # (n, s, i)
    Y1 = Y[:, :, 1, :]  # (n, s, j)
    Z2 = Y[:, :, 2, :] + (S + bias.reshape(1, OUT_DIM))[:, :, None]  # (n, s, k)
    W = (Y1[:, :, :, None] + Z2[:, :, None, :]).reshape(N_BATCH, OUT_DIM, JK)
    return W.astype(np.float16), A.astype(np.float32)


def _make_in_maps(W, A):
    in_maps = []
    for c in range(N_CORES):
        n = c // 2
        i0 = (c % 2) * I_PER_CORE
        w128 = W[n].reshape(128, PITCH)
        a_in = (
            A[n, :, i0 : i0 + I_PER_CORE]
            .reshape(OUT_DIM, N_CHUNKS, I_CHUNK)
            .transpose(0, 2, 1)
            .reshape(128, N_CHUNKS)
        )
        in_maps.append(
            {"w": np.ascontiguousarray(w128), "a": np.ascontiguousarray(a_in)}
        )
    return in_maps


def _run(inputs, trace=False, **kwargs):
    W, A = _host_precompute(inputs["x"], inputs["coefs"], inputs["bias"])
    if "nc" not in _PROGRAM_CACHE:
        _PROGRAM_CACHE["nc"] = _build_program()
    nc = _PROGRAM_CACHE["nc"]
    in_maps = _make_in_maps(W, A)
    res = run_bass_kernel_spmd(nc, in_maps, list(range(N_CORES)), trace=trace, **kwargs)

    out = np.empty((N_BATCH, OUT_DIM, M, M, M), dtype=np.float32)
    for c in range(N_CORES):
        n = c // 2
        i0 = (c % 2) * I_PER_CORE
        blk = res.results[c]["o"].astype(np.float32).reshape(
            N_CHUNKS, OUT_DIM, I_CHUNK, M, M
        )
        out[n, :, i0 : i0 + I_PER_CORE] = blk.transpose(1, 0, 2, 3, 4).reshape(
            OUT_DIM, I_PER_CORE, M, M
        )
    return out, res


def kernel(**inputs) -> np.ndarray:
    out, _ = _run(inputs, trace=False)
    return out


if __name__ == "__main__":
    rng = np.random.default_rng(0)
    x = rng.standard_normal((N_BATCH, IN_DIM, M), dtype=np.float32)
    coefs = rng.standard_normal((IN_DIM, OUT_DIM, 4), dtype=np.float32)
    bias = np.zeros((1, OUT_DIM, 1, 1, 1), dtype=np.float32)
    out = kernel(x=x, coefs=coefs, bias=bias)
    # host reference for smoke check
    Y = np.einsum("ndi,dsb->nsbi", x, coefs[:, :, :3])
    S = np.einsum("nd,ds->ns", x.sum(-1), coefs[:, :, 3])
    exp = (
        Y[:, :, 0, :, None, None]
        + Y[:, :, 1, None, :, None]
        + Y[:, :, 2, None, None, :]
        + S[:, :, None, None, None]
    )
    err = float(np.abs(out - exp).max() / np.abs(exp).max())
    print("smoke rel err:", err)


# revision 4
# speedup vs baseline: 1.4656x; 1.4656x over previous
"""Trainium2 Bass kernel for nn_Eq1to3 (gnn_message_passing).

Reference computation:
    Y  = einsum('ndi,dsb->nsbi', x, coefs[:, :, :3])      # (n, s, 3, m)
    S  = einsum('nd,ds->ns', x.sum(-1), coefs[:, :, 3])   # (n, s)
    out[n,s,i,j,k] = Y0[n,s,i] + Y1[n,s,j] + Y2[n,s,k] + S[n,s] + bias[s]

Shapes: x (4, 16, 96) f32 -> out (4, 16, 96, 96, 96) f32 (~226.5 MB).
The contractions are tiny; the kernel is output-write bound.

Strategy (8 NeuronCores):
  * Shard (n, i): core c handles n = c//2, i in [48*(c%2), 48*(c%2)+48).
    Per-core output 7.08M elements - balanced, no collectives.
  * Host precomputes (microscopic contractions, fp32 exact):
        W[n, s, (j,k)] = Y1[n,s,j] + Y2[n,s,k] + S[n,s] + bias[s]   (i-free)
        A[n, s, i]     = Y0[n,s,i]
    and ships W as fp16 (the device output is fp16; see below).
  * Device tile layout: 128 partitions = (s: 16) x (i-chunk: 8), free dim =
    (j,k) = 9216.  big0 (fp16) holds W replicated 8x per s-row, built once
    by 8 zero-stride broadcast DMAs from the packed (128, 1152) fp16 w.
  * Per i-chunk t (6 total): big = big0 + a_t with the per-element adds
    split across TWO engines by clock ratio - DVE (tensor_scalar_add) on
    cols [0:4096], ACT (activation Identity w/ per-partition bias) on cols
    [4096:9216] - then each engine half is DMA'd to HBM from its own HWDGE
    ring (SP / ACT) as soon as its sub-ops finish.
  * fp16 output: halves HBM write bytes (14.2 MB/core).  Total device-side
    rounding error ~1e-3 relative (fp16 W storage + fp16 output rounding)
    vs the 2e-2 gate.  Host casts back to f32 on reassembly.

Measured on HW (in-NEFF replication slope, K=1 vs 201, interleaved
single-dispatch sampling): ~17-20 us/exec vs ~147 us for the previous
all-f32 single-engine version.  Only SP/ACT can host HWDGE rings; 1-queue
vs 2-queue f32 probes showed no queue-count scaling, while fp16 halves
bytes ~2.5x faster => the win is bytes + descriptor shape, not queues.
"""

import dataclasses
import sys

sys.path.insert(0, "/opt/trn_rl_repo")

import numpy as np

import concourse.bacc as bacc
import concourse.mybir as mybir
from concourse.tile import TileContext
from concourse.bass_utils import run_bass_kernel_spmd

N_BATCH = 4
IN_DIM = 16
OUT_DIM = 16
M = 96
JK = M * M  # 9216
N_CORES = 8
I_PER_CORE = 48  # one n, half of the i axis per core
I_CHUNK = 8  # 16 s * 8 i' = 128 partitions
N_CHUNKS = I_PER_CORE // I_CHUNK  # 6
PITCH = JK // I_CHUNK  # 1152: packed-W row length
# DVE runs packed-fp16 tensor_scalar at 4 elem/cycle/lane (4x_2p perf mode,
# confirmed in the cost model and on HW), so it takes most columns:
# 7680/3.84GHz ~= 2 us/chunk vs ACT's 1536/1.2GHz ~= 1.3 us/chunk.
C_DVE = 7680  # DVE's columns per chunk; ACT takes the rest
DVE_SPLIT = 5  # 1536-col sub-ops; first 3 cover the sync-ring DMA half
ACT_SPLIT = 2  # 768-col sub-ops
C_RING = 4608  # ring byte-balance boundary: sync [0:4608], scalar [4608:]

_PROGRAM_CACHE = {}


def _build_program(rep=1, serialize=False):
    """rep>1 replicates the kernel body in-NEFF for slope timing (test.py);
    serialize chains rep r+1's compute to a DRAM readback of rep r's last
    output chunk so repeats can neither overlap nor be elided.  The
    production program is rep=1, serialize=False."""
    nc = bacc.Bacc(None)
    # Packed W: row p = W[n, p//8, (p%8)*PITCH : (p%8+1)*PITCH]  (128, 1152)
    w_d = nc.dram_tensor("w", [128, PITCH], mybir.dt.float16, kind="ExternalInput")
    # A columns: a[p, t] = Y0 value for partition p = (s, i') in i-chunk t
    a_d = nc.dram_tensor("a", [128, N_CHUNKS], mybir.dt.float32, kind="ExternalInput")
    o_d = nc.dram_tensor(
        "o", [N_CHUNKS, OUT_DIM, I_CHUNK, JK], mybir.dt.float16, kind="ExternalOutput"
    )

    with TileContext(nc) as tc:
        with (
            tc.tile_pool(name="spool", bufs=2) as spool,
            tc.tile_pool(name="b0pool", bufs=1) as b0pool,
            tc.tile_pool(name="bigpool", bufs=4) as bigpool,
        ):
            big0 = b0pool.tile([128, JK], mybir.dt.float16)
            # Replicate: big0[p=(s,i'), e*PITCH+k'] = w[s*8+e, k'] for all
            # i'.  DRAM source AP [[PITCH*8, 16], [0, 8], [1, PITCH]] at
            # offset e*PITCH: the zero-stride middle dim re-reads each packed
            # W row for all 8 destination partitions of its s-group.
            for e in range(I_CHUNK):
                src = dataclasses.replace(
                    w_d[:],
                    offset=e * PITCH,
                    ap=[[PITCH * I_CHUNK, OUT_DIM], [0, I_CHUNK], [1, PITCH]],
                )
                eng = nc.sync if e % 2 == 0 else nc.scalar
                eng.dma_start(out=big0[:, e * PITCH : (e + 1) * PITCH], in_=src)

            for r in range(rep):
                a_sb = spool.tile([128, N_CHUNKS], mybir.dt.float32)
                nc.sync.dma_start(out=a_sb[:], in_=a_d[:])
                if serialize and r > 0:
                    # RAW through DRAM (timing rig only): rep r's compute
                    # consumes bytes DMA'd back from rep r-1's last output
                    # chunk.  Corrupts big0[:, 0:1]; values are irrelevant.
                    rb = spool.tile([128, 1], mybir.dt.float16)
                    nc.sync.dma_start(out=rb[:, 0:1], in_=o_d[5, :, 0, 0:8])
                    nc.vector.tensor_copy(out=big0[:, 0:1], in_=rb[:, 0:1])
                for t in range(N_CHUNKS):
                    big = bigpool.tile([128, JK], mybir.dt.float16)
                    a_t = a_sb[:, t : t + 1]
                    fs = C_DVE // DVE_SPLIT
                    for f in range(DVE_SPLIT):
                        sl = slice(f * fs, (f + 1) * fs)
                        nc.vector.tensor_scalar_add(
                            out=big[:, sl], in0=big0[:, sl], scalar1=a_t
                        )
                    fs2 = (JK - C_DVE) // ACT_SPLIT
                    for f in range(ACT_SPLIT):
                        sl = slice(C_DVE + f * fs2, C_DVE + (f + 1) * fs2)
                        nc.scalar.add(out=big[:, sl], in_=big0[:, sl], add=a_t)
                    # Ring split != compute split: keep the two HWDGE rings
                    # byte-balanced (4608 cols each).
                    nc.sync.dma_start(out=o_d[t, :, :, 0:C_RING], in_=big[:, 0:C_RING])
                    nc.scalar.dma_start(
                        out=o_d[t, :, :, C_RING:C_DVE], in_=big[:, C_RING:C_DVE]
                    )
                    nc.scalar.dma_start(
                        out=o_d[t, :, :, C_DVE:JK], in_=big[:, C_DVE:JK]
                    )
    nc.compile()
    return nc


def _host_precompute(x, coefs, bias):
    x = np.asarray(x, dtype=np.float32)
    coefs = np.asarray(coefs, dtype=np.float32)
    bias = np.asarray(bias, dtype=np.float32)
    Y = np.einsum("ndi,dsb->nsbi", x, coefs[:, :, :3], optimize=True).astype(np.float32)
    S = np.einsum("nd,ds->ns", x.sum(axis=-1), coefs[:, :, 3], optimize=True).astype(
        np.float32
    )
    A = Y[:, :, 0, :]  # (n, s, i)
    Y1 = Y[:, :, 1, :]  # (n, s, j)
    Z2 = Y[:, :, 2, :] + (S + bias.reshape(1, OUT_DIM))[:, :, None]  # (n, s, k)
    W = (Y1[:, :, :, None] + Z2[:, :, None, :]).reshape(N_BATCH, OUT_DIM, JK)
    return W.astype(np.float16), A.astype(np.float32)


def _make_in_maps(W, A):
    in_maps = []
    for c in range(N_CORES):
        n = c // 2
        i0 = (c % 2) * I_PER_CORE
        w128 = W[n].reshape(128, PITCH)
        a_in = (
            A[n, :, i0 : i0 + I_PER_CORE]
            .reshape(OUT_DIM, N_CHUNKS, I_CHUNK)
            .transpose(0, 2, 1)
            .reshape(128, N_CHUNKS)
        )
        in_maps.append(
            {"w": np.ascontiguousarray(w128), "a": np.ascontiguousarray(a_in)}
        )
    return in_maps


def _run(inputs, trace=False, **kwargs):
    W, A = _host_precompute(inputs["x"], inputs["coefs"], inputs["bias"])
    if "nc" not in _PROGRAM_CACHE:
        _PROGRAM_CACHE["nc"] = _build_program()
    nc = _PROGRAM_CACHE["nc"]
    in_maps = _make_in_maps(W, A)
    res = run_bass_kernel_spmd(nc, in_maps, list(range(N_CORES)), trace=trace, **kwargs)

    out = np.empty((N_BATCH, OUT_DIM, M, M, M), dtype=np.float32)
    for c in range(N_CORES):
        n = c // 2
        i0 = (c % 2) * I_PER_CORE
        blk = res.results[c]["o"].astype(np.float32).reshape(
            N_CHUNKS, OUT_DIM, I_CHUNK, M, M
        )
        out[n, :, i0 : i0 + I_PER_CORE] = blk.transpose(1, 0, 2, 3, 4).reshape(
            OUT_DIM, I_PER_CORE, M, M
        )
    return out, res


def kernel(**inputs) -> np.ndarray:
    out, _ = _run(inputs, trace=False)
    return out


if __name__ == "__main__":
    rng = np.random.default_rng(0)
    x = rng.standard_normal((N_BATCH, IN_DIM, M), dtype=np.float32)
    coefs = rng.standard_normal((IN_DIM, OUT_DIM, 4), dtype=np.float32)
    bias = np.zeros((1, OUT_DIM, 1, 1, 1), dtype=np.float32)
    out = kernel(x=x, coefs=coefs, bias=bias)
    # host reference for smoke check
    Y = np.einsum("ndi,dsb->nsbi", x, coefs[:, :, :3])
    S = np.einsum("nd,ds->ns", x.sum(-1), coefs[:, :, 3])
    exp = (
        Y[:, :, 0, :, None, None]
        + Y[:, :, 1, None, :, None]
        + Y[:, :, 2, None, None, :]
        + S[:, :, None, None, None]
    )
    err = float(np.abs(out - exp).max() / np.abs(exp).max())
    print("smoke rel err:", err)
